# revision 3
# baseline (speedup 1.0000x reference)
# Depthwise 4x4 conv (DiagonalwiseRefactorization) on 8 TRN2 NeuronCores.
#
# The mask zeroes every weight except weight[c, c % 64], and with
# feature_group_count=8 the grouped conv collapses to a depthwise conv:
#   out[n, c, ho, wo] = sum_{kh, kw} w[c, kh, kw] * xpad[n, c, ho+kh, wo+kw]
# with pad=1, stride=1: (16, 512, 64, 64) -> (16, 512, 63, 63).
#
# Device strategy (per core: 64 channels x 16 images, no inter-core comm):
#   For each width-tap kw, the H-direction conv is a banded-Toeplitz matmul:
#     out[c, :, wo] += T_c_kw.T @ xrow[c, :, wo+kw]
#   where T_c_kw[h, ho] = w[c, h-ho+1, kw] (64x64, 4 diagonals). Two channels
#   share one 128x128 block-diagonal stationary operand; the 4 kw taps
#   accumulate in PSUM. All matmuls are bf16 with fp32 PSUM accumulation.
#
# Host does layout only: bf16 cast + swizzle of x to [pair, c'*64+h, n, wpad],
# building the block-diag lhsT tensors from (weight*mask), and un-permuting
# the swizzled fp32 output.

import sys
import types

import numpy as np
import ml_dtypes

BF16 = ml_dtypes.bfloat16

N_CORES = 8
IMGS = 16
CH_TOT = 512
CH = CH_TOT // N_CORES  # 64 channels per core
PAIRS = CH // 2  # 32
H = W = 64
HO = WO = 63
WPAD = W + 2  # 66 (one zero column each side)
NHALF = IMGS // 2  # 8 images per psum tile
NFREE = NHALF * WO  # 504 <= 512 (one PSUM bank)


def _install_axon_hooks_shim():
    """Make trace=True work under axon: bass_utils imports
    antenv.axon_hooks, which the container's antenv stub lacks."""
    try:
        import antenv.axon_hooks  # noqa: F401

        return
    except ImportError:
        pass
    try:
        import antenv
    except ImportError:
        return
    mod = types.ModuleType("antenv.axon_hooks")
    mod._hook = None

    def set_axon_ntff_profile_hook(h):
        mod._hook = h

    def get_axon_ntff_profile_hook():
        return mod._hook

    mod.set_axon_ntff_profile_hook = set_axon_ntff_profile_hook
    mod.get_axon_ntff_profile_hook = get_axon_ntff_profile_hook
    sys.modules["antenv.axon_hooks"] = mod
    antenv.axon_hooks = mod
    try:
        from trn_agent_boot.trn_boot import _ntff_profile_via_ctypes

        hook = _ntff_profile_via_ctypes("/opt/axon/libaxon_pjrt.so")
        if hook is not None:
            mod._hook = hook
    except Exception:
        pass


_install_axon_hooks_shim()

import concourse.bacc as bacc  # noqa: E402
import concourse.mybir as mybir  # noqa: E402
import concourse.tile as tile  # noqa: E402
from concourse.bass_utils import run_bass_kernel_spmd  # noqa: E402

LAST_RESULT = None
_NC_CACHE = None


def _build_nc():
    nc = bacc.Bacc("TRN2", target_bir_lowering=False, debug=False, num_devices=N_CORES)

    xd = nc.dram_tensor(
        "xin", [PAIRS, 128, IMGS, WPAD], mybir.dt.bfloat16, kind="ExternalInput"
    )
    wd = nc.dram_tensor(
        "wt", [PAIRS, 128, 4, 128], mybir.dt.bfloat16, kind="ExternalInput"
    )
    od = nc.dram_tensor(
        "out", [PAIRS, 2, 128, NFREE], mybir.dt.float32, kind="ExternalOutput"
    )

    with tile.TileContext(nc) as tc:
        with (
            tc.tile_pool(name="xp", bufs=4) as xp,
            tc.tile_pool(name="wp", bufs=4) as wp,
            tc.tile_pool(name="ps", bufs=4, space="PSUM") as ps,
            tc.tile_pool(name="op", bufs=6) as op,
        ):
            for pair in range(PAIRS):
                xt = xp.tile([128, IMGS, WPAD], mybir.dt.bfloat16)
                nc.sync.dma_start(out=xt[:], in_=xd[pair])
                wtile = wp.tile([128, 4, 128], mybir.dt.bfloat16)
                nc.sync.dma_start(out=wtile[:], in_=wd[pair])

                pts = [
                    ps.tile([128, NFREE], mybir.dt.float32, name=f"pt{h}")
                    for h in range(2)
                ]
                for kw in range(4):
                    for half in range(2):
                        nc.tensor.matmul(
                            pts[half][:],
                            lhsT=wtile[:, kw, :],
                            rhs=xt[:, half * NHALF : (half + 1) * NHALF, kw : kw + WO],
                            start=(kw == 0),
                            stop=(kw == 3),
                        )
                for half in range(2):
                    ot = op.tile([128, NFREE], mybir.dt.float32)
                    nc.vector.tensor_copy(ot[:], pts[half][:])
                    nc.scalar.dma_start(out=od[pair, half], in_=ot[:])
    nc.compile()
    return nc


def _get_nc():
    global _NC_CACHE
    if _NC_CACHE is None:
        _NC_CACHE = _build_nc()
    return _NC_CACHE


def _prep_x(x):
    """x (16, 512, 64, 64) f32 -> per-core list of (PAIRS, 128, IMGS, WPAD) bf16.

    Partition index p = c'*64 + h for channel pair slot c' in {0, 1};
    free layout [n, 1 + w] with zero columns at w-offsets 0 and WPAD-1.
    """
    maps = []
    for k in range(N_CORES):
        xc = x[:, k * CH : (k + 1) * CH]  # (16, 64, 64, 64)
        t = xc.transpose(1, 2, 0, 3)  # (ch, h, n, w)
        arr = np.zeros((CH, H, IMGS, WPAD), dtype=BF16)
        arr[..., 1 : 1 + W] = t.astype(BF16)
        maps.append(np.ascontiguousarray(arr.reshape(PAIRS, 128, IMGS, WPAD)))
    return maps


def _prep_w(wc):
    """wc (512, 4, 4) f32 masked per-channel weights ->
    per-core (PAIRS, 128, 4, 128) bf16 block-diagonal banded lhsT.

    lhsT[pair, c'*64 + h, kw, c'*64 + ho] = wc[ch, h - ho + 1, kw]
    for 0 <= h - ho + 1 <= 3, ho <= 62 (columns 63 and 127 stay zero).
    """
    maps = []
    for k in range(N_CORES):
        wk = wc[k * CH : (k + 1) * CH]  # (64, 4, 4) [ch, kh, kw]
        blocks = np.zeros((CH, 4, H, H), dtype=np.float32)  # [ch, kw, h, ho]
        ho = np.arange(HO)
        for kh in range(4):
            h = ho + kh - 1
            v = (h >= 0) & (h < H)
            blocks[:, :, h[v], ho[v]] = wk[:, kh, :][:, :, None]
        br = blocks.reshape(PAIRS, 2, 4, H, H)  # [pair, c', kw, h, ho]
        lt6 = np.zeros((PAIRS, 2, H, 4, 2, H), dtype=np.float32)
        lt6[:, 0, :, :, 0, :] = br[:, 0].transpose(0, 2, 1, 3)  # [pair, h, kw, ho]
        lt6[:, 1, :, :, 1, :] = br[:, 1].transpose(0, 2, 1, 3)
        maps.append(np.ascontiguousarray(lt6.reshape(PAIRS, 128, 4, 128).astype(BF16)))
    return maps


def _unswizzle(out_dev):
    """(PAIRS, 2, 128, NFREE) f32 -> (16, 64, 63, 63) f32 for one core."""
    r = out_dev.reshape(PAIRS, 2, 2, H, NHALF, WO)  # [pair, half, c', ho64, n', wo]
    t = r.transpose(1, 4, 0, 2, 3, 5)  # [half, n', pair, c', ho64, wo]
    return np.ascontiguousarray(t.reshape(IMGS, CH, H, WO)[:, :, :HO, :])


def kernel(x, weight, mask, groups=8, stride=1, _trace=False, _trace_kwargs=None):
    global LAST_RESULT
    x = np.ascontiguousarray(np.asarray(x, dtype=np.float32))
    weight = np.asarray(weight, dtype=np.float32)
    mask = np.asarray(mask, dtype=np.float32)

    # Masked weights collapse to one 4x4 filter per output channel.
    wc = (weight * mask).sum(axis=1)  # (512, 4, 4)

    xs = _prep_x(x)
    ws = _prep_w(wc)
    in_maps = [{"xin": xs[k], "wt": ws[k]} for k in range(N_CORES)]

    nc = _get_nc()
    kwargs = {}
    if _trace:
        kwargs["trace"] = True
        if _trace_kwargs:
            kwargs.update(_trace_kwargs)
    res = run_bass_kernel_spmd(nc, in_maps, core_ids=list(range(N_CORES)), **kwargs)
    LAST_RESULT = res

    outs = [_unswizzle(res.results[k]["out"]) for k in range(N_CORES)]
    return np.concatenate(outs, axis=1)


def emulate(x, weight, mask, groups=8, stride=1):
    """Pure-numpy emulation of the device math (same bf16 rounding and
    packing) - validates host prep + Toeplitz construction without HW."""
    x = np.asarray(x, dtype=np.float32)
    wc = (np.asarray(weight, np.float32) * np.asarray(mask, np.float32)).sum(axis=1)
    xs = _prep_x(x)
    ws = _prep_w(wc)
    outs = []
    for k in range(N_CORES):
        xin = xs[k].astype(np.float32)  # (PAIRS, 128, IMGS, WPAD)
        wt = ws[k].astype(np.float32)  # (PAIRS, 128, 4, 128)
        out_dev = np.zeros((PAIRS, 2, 128, NFREE), dtype=np.float32)
        for pair in range(PAIRS):
            for half in range(2):
                acc = np.zeros((128, NFREE), dtype=np.float32)
                for kw in range(4):
                    rhs = xin[pair][:, half * NHALF : (half + 1) * NHALF, kw : kw + WO]
                    rhs = rhs.reshape(128, NFREE)
                    acc += wt[pair][:, kw, :].T @ rhs
                out_dev[pair, half] = acc
        outs.append(_unswizzle(out_dev))
    return np.concatenate(outs, axis=1)
